# revision 15
# baseline (speedup 1.0000x reference)
"""Block-diagonal Hamming-similarity gram kernel for TRN2 (8 NeuronCores).

Problem: x [B=4, N=1024, L=512, A=21] fp32; 16 consecutive groups of 64
sequences per batch; per group compute sims = (Xg @ Xg.T) / L where Xg is
[64, L*A=10752]; output [B, N, N] is NaN-filled with the 64x64 blocks
written on the block diagonal.

Strategy:
- Shard the 64 independent (batch, group) blocks across 8 cores (8 each).
- Host marshals each block to a feature-major fp16 layout
  xt[t, p, c*64+n] = x[b, g*64+n, c*128+p], so every 128-row contraction
  chunk is a ready-to-use [K=128, 64] SBUF tile for both matmul operands
  (lhsT == rhs -> gram), with fully contiguous per-partition DMA.
- fp16 runs the PE at 1 cycle/row (fp32 is 4) and halves DMA bytes;
  measured accuracy vs the fp32 reference is ~2.4e-5 relative.
- Per block: accumulate 84 chunk matmuls into one PSUM [64, 64], scale by
  1/L during the PSUM->SBUF copy, DMA out. NaN canvas + block scatter on
  host.
"""

import numpy as np

B, N, L, A = 4, 1024, 512, 21
G, GS = 16, 64
LA = L * A           # 10752
K = 128              # contraction tile (partition dim)
C = LA // K          # 84 chunks
NBLK = B * G         # 64 independent gram blocks
NCORES = 8
BPC = NBLK // NCORES # 8 blocks per core

_CACHE = {}


def _build_nc():
    if "nc" in _CACHE:
        return _CACHE["nc"]
    import concourse.mybir as mybir
    from concourse import bacc
    from concourse.tile import TileContext

    # Input DMA slicing: pairs first (stream efficiency), single blocks last
    # so the final block is available as early as possible. 5 input + 1
    # output DMA = 6 HWDGE sem lanes; with PE + DVE that is 8 drain waits,
    # which still compiles.
    DMA_GROUPS = [[0, 1], [2, 3], [4, 5], [6], [7]]

    # Bacc (not raw Bass): its finalize() legalizes multi-sem waits into
    # event-semaphore trees, which the TRN2 ISA requires (1 wait/inst).
    nc = bacc.Bacc("TRN2", target_bir_lowering=False)
    xt = nc.declare_dram_parameter("xt", [BPC, K, C * GS], mybir.dt.float16, isOutput=False)
    sims = nc.declare_dram_parameter("sims", [GS, BPC * GS], mybir.dt.float32, isOutput=True)

    with TileContext(nc) as tc:
        with (
            tc.tile_pool(name="xin", bufs=len(DMA_GROUPS)) as xpool,
            tc.tile_pool(name="ps", bufs=8, space="PSUM") as ppool,
            tc.tile_pool(name="osb", bufs=1) as opool,
        ):
            block_slice = {}
            for grp in DMA_GROUPS:
                n = len(grp)
                xtile = xpool.tile([K, n * C * GS], mybir.dt.float16,
                                   tag=f"x{n}")
                src = xt[grp[0]:grp[0] + n].rearrange("g p f -> p g f")
                dst = xtile.rearrange("p (g f) -> p g f", g=n)
                nc.sync.dma_start(out=dst, in_=src)
                for j, g in enumerate(grp):
                    block_slice[g] = (xtile, j * C * GS)
            ob = opool.tile([GS, BPC * GS], mybir.dt.float32, tag="o")
            NP = C // 2  # 42 chunk pairs
            for g in range(BPC):
                xtile, base = block_slice[g]
                # Chunk-pair matmuls: stationary = [128, 128] (two adjacent
                # 128-row contraction chunks side by side) -> fast weight
                # load; each of the two moving halves contributes its gram to
                # one 64-partition half of a PSUM tile (the other half is
                # discarded cross-chunk garbage).
                psA = ppool.tile([2 * GS, GS], mybir.dt.float32, tag="ps")
                psB = ppool.tile([2 * GS, GS], mybir.dt.float32, tag="ps")
                for p in range(NP):
                    pair = xtile[:, base + p * 2 * GS: base + (p + 1) * 2 * GS]
                    rhs_e = xtile[:, base + p * 2 * GS: base + p * 2 * GS + GS]
                    rhs_o = xtile[:, base + p * 2 * GS + GS: base + (p + 1) * 2 * GS]
                    nc.tensor.matmul(psA, lhsT=pair, rhs=rhs_e,
                                     start=(p == 0), stop=(p == NP - 1))
                    nc.tensor.matmul(psB, lhsT=pair, rhs=rhs_o,
                                     start=(p == 0), stop=(p == NP - 1))
                # gram = even-chunk sum (psA rows 0:64) + odd-chunk sum
                # (psB rows 64:128); input is pre-scaled by 1/sqrt(L) on the
                # host so no further scaling is needed. Two DVE ops because
                # only one non-scalar operand may come from PSUM.
                oslice = ob[:, g * GS:(g + 1) * GS]
                nc.vector.tensor_copy(oslice, psA[0:GS, :])
                nc.vector.scalar_tensor_tensor(
                    oslice,
                    oslice,
                    1.0,
                    psB[GS:2 * GS, :],
                    op0=mybir.AluOpType.mult,
                    op1=mybir.AluOpType.add,
                )
            nc.sync.dma_start(out=sims[:], in_=ob)

    nc.finalize()
    _CACHE["nc"] = nc
    return nc


def _marshal(x):
    # [B,N,L,A] -> blocks [NBLK, K, C*GS] fp16, feature-major per block.
    # Pre-scaled by 1/sqrt(L) so the on-device gram needs no 1/L scaling.
    xs = np.asarray(x, dtype=np.float32).reshape(B, G, GS, C, K)
    xs = xs * np.float32(1.0 / np.sqrt(L))
    xt = xs.transpose(0, 1, 4, 3, 2)                 # [b, g, p, c, n]
    return np.ascontiguousarray(xt, dtype=np.float16).reshape(NBLK, K, C * GS)


def run(x, trace=False):
    from concourse.bass_utils import run_bass_kernel_spmd

    nc = _build_nc()
    xt16 = _marshal(x)
    in_maps = [{"xt": xt16[m * BPC:(m + 1) * BPC]} for m in range(NCORES)]
    res = run_bass_kernel_spmd(nc, in_maps, list(range(NCORES)), trace=trace)
    # per-core result is [GS, BPC*GS] = [i, (g j)] -> [g, i, j]
    sims = np.stack(
        [
            res.results[m]["sims"].reshape(GS, BPC, GS).transpose(1, 0, 2)
            for m in range(NCORES)
        ],
    ).reshape(B, G, GS, GS)

    out = np.full((B, N, N), np.nan, dtype=np.float32)
    for g in range(G):
        out[:, g * GS:(g + 1) * GS, g * GS:(g + 1) * GS] = sims[:, g]
    return out, res


def kernel(x):
    out, _ = run(x, trace=False)
    return out


# revision 16
# speedup vs baseline: 1.0632x; 1.0632x over previous
"""Block-diagonal Hamming-similarity gram kernel for TRN2 (8 NeuronCores).

Problem: x [B=4, N=1024, L=512, A=21] fp32; 16 consecutive groups of 64
sequences per batch; per group compute sims = (Xg @ Xg.T) / L where Xg is
[64, L*A=10752]; output [B, N, N] is NaN-filled with the 64x64 blocks
written on the block diagonal.

Strategy:
- Shard the 64 independent (batch, group) blocks across 8 cores (8 each).
- Host marshals each block to a feature-major fp16 layout
  xt[t, p, c*64+n] = x[b, g*64+n, c*128+p], so every 128-row contraction
  chunk is a ready-to-use [K=128, 64] SBUF tile for both matmul operands
  (lhsT == rhs -> gram), with fully contiguous per-partition DMA.
- fp16 runs the PE at 1 cycle/row (fp32 is 4) and halves DMA bytes;
  measured accuracy vs the fp32 reference is ~2.4e-5 relative.
- Per block: accumulate 84 chunk matmuls into one PSUM [64, 64], scale by
  1/L during the PSUM->SBUF copy, DMA out. NaN canvas + block scatter on
  host.
"""

import numpy as np

B, N, L, A = 4, 1024, 512, 21
G, GS = 16, 64
LA = L * A           # 10752
K = 128              # contraction tile (partition dim)
C = LA // K          # 84 chunks
NBLK = B * G         # 64 independent gram blocks
NCORES = 8
BPC = NBLK // NCORES # 8 blocks per core

_CACHE = {}


def _build_nc():
    if "nc" in _CACHE:
        return _CACHE["nc"]
    import concourse.mybir as mybir
    from concourse import bacc
    from concourse.tile import TileContext

    # Input DMA slicing: pairs first (stream efficiency), single blocks last
    # so the final block is available as early as possible. 5 input + 1
    # output DMA = 6 HWDGE sem lanes; with PE + DVE that is 8 drain waits,
    # which still compiles.
    DMA_GROUPS = [[0, 1], [2, 3], [4, 5], [6], [7]]

    # Bacc (not raw Bass): its finalize() legalizes multi-sem waits into
    # event-semaphore trees, which the TRN2 ISA requires (1 wait/inst).
    nc = bacc.Bacc("TRN2", target_bir_lowering=False)
    xt = nc.declare_dram_parameter("xt", [BPC, K, C * GS], mybir.dt.float16, isOutput=False)
    sims = nc.declare_dram_parameter("sims", [GS, BPC * GS], mybir.dt.float32, isOutput=True)

    with TileContext(nc) as tc:
        with (
            tc.tile_pool(name="xin", bufs=len(DMA_GROUPS)) as xpool,
            tc.tile_pool(name="ps", bufs=8, space="PSUM") as ppool,
            tc.tile_pool(name="osb", bufs=1) as opool,
        ):
            block_slice = {}
            for grp in DMA_GROUPS:
                n = len(grp)
                xtile = xpool.tile([K, n * C * GS], mybir.dt.float16,
                                   tag=f"x{n}")
                src = xt[grp[0]:grp[0] + n].rearrange("g p f -> p g f")
                dst = xtile.rearrange("p (g f) -> p g f", g=n)
                nc.sync.dma_start(out=dst, in_=src)
                for j, g in enumerate(grp):
                    block_slice[g] = (xtile, j * C * GS)
            ob = opool.tile([GS, BPC * GS], mybir.dt.float32, tag="o")
            NP = C // 2  # 42 chunk pairs
            for g in range(BPC):
                xtile, base = block_slice[g]
                # Chunk-pair matmul: stationary AND moving are the [128, 128]
                # pair of adjacent contraction chunks -> one LDWEIGHTS + one
                # MATMUL per two chunks. The [128,128] product holds the
                # even-chunk gram in quadrant [0:64, 0:64] and the odd-chunk
                # gram in [64:128, 64:128]; the off-diagonal quadrants are
                # discarded cross-chunk garbage.
                ps = ppool.tile([2 * GS, 2 * GS], mybir.dt.float32, tag="ps")
                for p in range(NP):
                    pair = xtile[:, base + p * 2 * GS: base + (p + 1) * 2 * GS]
                    nc.tensor.matmul(ps, lhsT=pair, rhs=pair,
                                     start=(p == 0), stop=(p == NP - 1))
                # gram = even-chunk sum + odd-chunk sum; input is pre-scaled
                # by 1/sqrt(L) on the host so no further scaling is needed.
                # Two DVE ops because only one operand may come from PSUM.
                oslice = ob[:, g * GS:(g + 1) * GS]
                nc.vector.tensor_copy(oslice, ps[0:GS, 0:GS])
                nc.vector.scalar_tensor_tensor(
                    oslice,
                    oslice,
                    1.0,
                    ps[GS:2 * GS, GS:2 * GS],
                    op0=mybir.AluOpType.mult,
                    op1=mybir.AluOpType.add,
                )
            nc.sync.dma_start(out=sims[:], in_=ob)

    nc.finalize()
    _CACHE["nc"] = nc
    return nc


def _marshal(x):
    # [B,N,L,A] -> blocks [NBLK, K, C*GS] fp16, feature-major per block.
    # Pre-scaled by 1/sqrt(L) so the on-device gram needs no 1/L scaling.
    xs = np.asarray(x, dtype=np.float32).reshape(B, G, GS, C, K)
    xs = xs * np.float32(1.0 / np.sqrt(L))
    xt = xs.transpose(0, 1, 4, 3, 2)                 # [b, g, p, c, n]
    return np.ascontiguousarray(xt, dtype=np.float16).reshape(NBLK, K, C * GS)


def run(x, trace=False):
    from concourse.bass_utils import run_bass_kernel_spmd

    nc = _build_nc()
    xt16 = _marshal(x)
    in_maps = [{"xt": xt16[m * BPC:(m + 1) * BPC]} for m in range(NCORES)]
    res = run_bass_kernel_spmd(nc, in_maps, list(range(NCORES)), trace=trace)
    # per-core result is [GS, BPC*GS] = [i, (g j)] -> [g, i, j]
    sims = np.stack(
        [
            res.results[m]["sims"].reshape(GS, BPC, GS).transpose(1, 0, 2)
            for m in range(NCORES)
        ],
    ).reshape(B, G, GS, GS)

    out = np.full((B, N, N), np.nan, dtype=np.float32)
    for g in range(G):
        out[:, g * GS:(g + 1) * GS, g * GS:(g + 1) * GS] = sims[:, g]
    return out, res


def kernel(x):
    out, _ = run(x, trace=False)
    return out


# revision 23
# speedup vs baseline: 1.1430x; 1.0751x over previous
"""Block-diagonal Hamming-similarity gram kernel for TRN2 (8 NeuronCores).

Problem: x [B=4, N=1024, L=512, A=21] fp32; 16 consecutive groups of 64
sequences per batch; per group compute sims = (Xg @ Xg.T) / L where Xg is
[64, L*A=10752]; output [B, N, N] is NaN-filled with the 64x64 blocks
written on the block diagonal.

Strategy:
- Shard the 64 independent (batch, group) blocks across 8 cores (8 each).
- Host marshals each block to a feature-major fp16 layout
  xt[t, p, c*64+n] = x[b, g*64+n, c*128+p], so every 128-row contraction
  chunk is a ready-to-use [K=128, 64] SBUF tile for both matmul operands
  (lhsT == rhs -> gram), with fully contiguous per-partition DMA.
- fp16 runs the PE at 1 cycle/row (fp32 is 4) and halves DMA bytes;
  measured accuracy vs the fp32 reference is ~2.4e-5 relative.
- Per block: accumulate 84 chunk matmuls into one PSUM [64, 64], scale by
  1/L during the PSUM->SBUF copy, DMA out. NaN canvas + block scatter on
  host.
"""

import numpy as np

B, N, L, A = 4, 1024, 512, 21
G, GS = 16, 64
LA = L * A           # 10752
K = 128              # contraction tile (partition dim)
C = LA // K          # 84 chunks
NBLK = B * G         # 64 independent gram blocks
NCORES = 8
BPC = NBLK // NCORES # 8 blocks per core

_CACHE = {}


def _build_nc():
    if "nc" in _CACHE:
        return _CACHE["nc"]
    import concourse.mybir as mybir
    from concourse import bacc
    from concourse.tile import TileContext

    # Input DMA slicing: pairs first (stream efficiency), single blocks last
    # so the final block is available as early as possible. 5 input + 1
    # output DMA = 6 HWDGE sem lanes; with PE + DVE that is 8 drain waits,
    # which still compiles.
    DMA_GROUPS = [[0, 1], [2, 3], [4, 5], [6], [7]]

    # Bacc (not raw Bass): its finalize() legalizes multi-sem waits into
    # event-semaphore trees, which the TRN2 ISA requires (1 wait/inst).
    nc = bacc.Bacc("TRN2", target_bir_lowering=False)
    xt = nc.declare_dram_parameter("xt", [BPC, K, C * GS], mybir.dt.float16, isOutput=False)
    sims = nc.declare_dram_parameter("sims", [GS, BPC * GS], mybir.dt.float32, isOutput=True)

    with TileContext(nc) as tc:
        with (
            tc.tile_pool(name="xin", bufs=len(DMA_GROUPS)) as xpool,
            tc.tile_pool(name="ps", bufs=8, space="PSUM") as ppool,
            tc.tile_pool(name="osb", bufs=1) as opool,
        ):
            block_slice = {}
            for grp in DMA_GROUPS:
                n = len(grp)
                xtile = xpool.tile([K, n * C * GS], mybir.dt.float16,
                                   tag=f"x{n}")
                src = xt[grp[0]:grp[0] + n].rearrange("g p f -> p g f")
                dst = xtile.rearrange("p (g f) -> p g f", g=n)
                nc.sync.dma_start(out=dst, in_=src)
                for j, g in enumerate(grp):
                    block_slice[g] = (xtile, j * C * GS)
            ob = opool.tile([GS, BPC * GS], mybir.dt.float32, tag="o")
            NP = C // 2  # 42 chunk pairs
            for g in range(BPC):
                xtile, base = block_slice[g]
                # Chunk-pair matmul: stationary AND moving are the [128, 128]
                # pair of adjacent contraction chunks -> one LDWEIGHTS + one
                # MATMUL per two chunks. The [128,128] product holds the
                # even-chunk gram in quadrant [0:64, 0:64] and the odd-chunk
                # gram in [64:128, 64:128]; the off-diagonal quadrants are
                # discarded cross-chunk garbage.
                ps = ppool.tile([2 * GS, 2 * GS], mybir.dt.float32, tag="ps")
                for p in range(NP):
                    pair = xtile[:, base + p * 2 * GS: base + (p + 1) * 2 * GS]
                    nc.tensor.matmul(ps, lhsT=pair, rhs=pair,
                                     start=(p == 0), stop=(p == NP - 1))
                # gram = even-chunk sum + odd-chunk sum; input is pre-scaled
                # by 1/sqrt(L) on the host so no further scaling is needed.
                # Two DVE ops because only one operand may come from PSUM.
                oslice = ob[:, g * GS:(g + 1) * GS]
                nc.vector.tensor_copy(oslice, ps[0:GS, 0:GS])
                nc.vector.scalar_tensor_tensor(
                    oslice,
                    oslice,
                    1.0,
                    ps[GS:2 * GS, GS:2 * GS],
                    op0=mybir.AluOpType.mult,
                    op1=mybir.AluOpType.add,
                )
            nc.sync.dma_start(out=sims[:], in_=ob)

    nc.finalize()
    _CACHE["nc"] = nc
    return nc


def _build_nc_raw():
    """Raw-bass variant: fine-grained streaming with manual semaphores.

    16 half-block input DMAs all increment ONE semaphore (HWDGE completes
    them in FIFO order), so the PE waits at half-block granularity and never
    idles long enough for the HAM clock gate to re-throttle. Two PSUM
    accumulators rotate per block; DVE combines the even/odd gram quadrants
    into the output tile; a single output DMA drains it at the end.
    """
    if "nc" in _CACHE:
        return _CACHE["nc"]
    import concourse.mybir as mybir
    from concourse import bacc

    NH = 2 * BPC                  # 16 half-blocks
    HF = (C // 2) * GS            # half-block free size: 42 chunks = 2688
    NPH = C // 4                  # 21 chunk-pairs per half

    nc = bacc.Bacc("TRN2", target_bir_lowering=False)
    xt = nc.declare_dram_parameter("xt", [BPC, K, C * GS], mybir.dt.float16, isOutput=False)
    sims = nc.declare_dram_parameter("sims", [GS, BPC * GS], mybir.dt.float32, isOutput=True)

    from contextlib import ExitStack
    with ExitStack() as ctx:
        xh = [ctx.enter_context(nc.sbuf_tensor(f"xh{h}", [K, HF], mybir.dt.float16))
              for h in range(NH)]
        ob = ctx.enter_context(nc.sbuf_tensor("ob", [GS, BPC * GS], mybir.dt.float32))
        ps = [ctx.enter_context(nc.psum_tensor(f"ps{i}", [2 * GS, 2 * GS], mybir.dt.float32))
              for i in range(2)]
        # One sem per input DMA: a DMA's +16 completion arrives as 16
        # separate +1s (one per SDMA engine), and engines skew across
        # pipelined DMAs, so a single shared counter is unsound.
        dma_sems = [ctx.enter_context(nc.semaphore(f"dma_in{h}"))
                    for h in range(NH)]
        pe_sem = ctx.enter_context(nc.semaphore("pe_done"))
        dve_sem = ctx.enter_context(nc.semaphore("dve_done"))
        out_sem = ctx.enter_context(nc.semaphore("dma_out"))

        with nc.Block() as block:

            @block.sync
            def _(sync):
                for h in range(NH):
                    g, j = divmod(h, 2)
                    src = xt[g][:, j * HF:(j + 1) * HF]
                    sync.dma_start(out=xh[h][:], in_=src).then_inc(dma_sems[h], 16)
                sync.wait_ge(dve_sem, BPC)
                sync.dma_start(out=sims[:], in_=ob[:]).then_inc(out_sem, 16)
                sync.wait_ge(out_sem, 16)
                # Leave all sems at 0 so the NEFF can be re-executed.
                for s in (*dma_sems, pe_sem, dve_sem, out_sem):
                    sync.sem_clear(s)

            @block.tensor
            def _(tensor):
                for g in range(BPC):
                    p = ps[g % 2]
                    if g >= 2:
                        # PSUM WAR: block g-2's epilogue must have read this
                        # accumulator before we clear it with start=True.
                        tensor.wait_ge(dve_sem, g - 1)
                    mm = None
                    for j in range(2):
                        h = 2 * g + j
                        tensor.wait_ge(dma_sems[h], 16)
                        for q in range(NPH):
                            pair = xh[h][:, q * 2 * GS:(q + 1) * 2 * GS]
                            pp = j * NPH + q
                            mm = nc.tensor.matmul(
                                p[:], lhsT=pair, rhs=pair,
                                start=(pp == 0), stop=(pp == 2 * NPH - 1))
                    mm.then_inc(pe_sem, 1)

            @block.vector
            def _(vector):
                for g in range(BPC):
                    p = ps[g % 2]
                    vector.wait_ge(pe_sem, g + 1)
                    oslice = ob[:, g * GS:(g + 1) * GS]
                    nc.vector.tensor_copy(oslice, p[0:GS, 0:GS])
                    nc.vector.scalar_tensor_tensor(
                        oslice, oslice, 1.0, p[GS:2 * GS, GS:2 * GS],
                        op0=mybir.AluOpType.mult,
                        op1=mybir.AluOpType.add,
                    ).then_inc(dve_sem, 1)

        nc.finalize()
    _CACHE["nc"] = nc
    return nc


def _marshal(x):
    # [B,N,L,A] -> blocks [NBLK, K, C*GS] fp16, feature-major per block.
    # Pre-scaled by 1/sqrt(L) so the on-device gram needs no 1/L scaling.
    xs = np.asarray(x, dtype=np.float32).reshape(B, G, GS, C, K)
    xs = xs * np.float32(1.0 / np.sqrt(L))
    xt = xs.transpose(0, 1, 4, 3, 2)                 # [b, g, p, c, n]
    return np.ascontiguousarray(xt, dtype=np.float16).reshape(NBLK, K, C * GS)


def run(x, trace=False):
    import os
    from concourse.bass_utils import run_bass_kernel_spmd

    nc = _build_nc() if os.environ.get("HS_TILE_KERNEL") else _build_nc_raw()
    xt16 = _marshal(x)
    in_maps = [{"xt": xt16[m * BPC:(m + 1) * BPC]} for m in range(NCORES)]
    res = run_bass_kernel_spmd(nc, in_maps, list(range(NCORES)), trace=trace)
    # per-core result is [GS, BPC*GS] = [i, (g j)] -> [g, i, j]
    sims = np.stack(
        [
            res.results[m]["sims"].reshape(GS, BPC, GS).transpose(1, 0, 2)
            for m in range(NCORES)
        ],
    ).reshape(B, G, GS, GS)

    out = np.full((B, N, N), np.nan, dtype=np.float32)
    for g in range(G):
        out[:, g * GS:(g + 1) * GS, g * GS:(g + 1) * GS] = sims[:, g]
    return out, res


def kernel(x):
    out, _ = run(x, trace=False)
    return out


# revision 24
# speedup vs baseline: 1.1589x; 1.0140x over previous
"""Block-diagonal Hamming-similarity gram kernel for TRN2 (8 NeuronCores).

Problem: x [B=4, N=1024, L=512, A=21] fp32; 16 consecutive groups of 64
sequences per batch; per group compute sims = (Xg @ Xg.T) / L where Xg is
[64, L*A=10752]; output [B, N, N] is NaN-filled with the 64x64 blocks
written on the block diagonal.

Strategy:
- Shard the 64 independent (batch, group) blocks across 8 cores (8 each).
- Host marshals each block to a feature-major fp16 layout
  xt[t, p, c*64+n] = x[b, g*64+n, c*128+p], so every 128-row contraction
  chunk is a ready-to-use [K=128, 64] SBUF tile for both matmul operands
  (lhsT == rhs -> gram), with fully contiguous per-partition DMA.
- fp16 runs the PE at 1 cycle/row (fp32 is 4) and halves DMA bytes;
  measured accuracy vs the fp32 reference is ~2.4e-5 relative.
- Per block: accumulate 84 chunk matmuls into one PSUM [64, 64], scale by
  1/L during the PSUM->SBUF copy, DMA out. NaN canvas + block scatter on
  host.
"""

import numpy as np

B, N, L, A = 4, 1024, 512, 21
G, GS = 16, 64
LA = L * A           # 10752
K = 128              # contraction tile (partition dim)
C = LA // K          # 84 chunks
NBLK = B * G         # 64 independent gram blocks
NCORES = 8
BPC = NBLK // NCORES # 8 blocks per core

_CACHE = {}


def _build_nc():
    if "nc" in _CACHE:
        return _CACHE["nc"]
    import concourse.mybir as mybir
    from concourse import bacc
    from concourse.tile import TileContext

    # Input DMA slicing: pairs first (stream efficiency), single blocks last
    # so the final block is available as early as possible. 5 input + 1
    # output DMA = 6 HWDGE sem lanes; with PE + DVE that is 8 drain waits,
    # which still compiles.
    DMA_GROUPS = [[0, 1], [2, 3], [4, 5], [6], [7]]

    # Bacc (not raw Bass): its finalize() legalizes multi-sem waits into
    # event-semaphore trees, which the TRN2 ISA requires (1 wait/inst).
    nc = bacc.Bacc("TRN2", target_bir_lowering=False)
    xt = nc.declare_dram_parameter("xt", [BPC, K, C * GS], mybir.dt.float16, isOutput=False)
    sims = nc.declare_dram_parameter("sims", [GS, BPC * GS], mybir.dt.float32, isOutput=True)

    with TileContext(nc) as tc:
        with (
            tc.tile_pool(name="xin", bufs=len(DMA_GROUPS)) as xpool,
            tc.tile_pool(name="ps", bufs=8, space="PSUM") as ppool,
            tc.tile_pool(name="osb", bufs=1) as opool,
        ):
            block_slice = {}
            for grp in DMA_GROUPS:
                n = len(grp)
                xtile = xpool.tile([K, n * C * GS], mybir.dt.float16,
                                   tag=f"x{n}")
                src = xt[grp[0]:grp[0] + n].rearrange("g p f -> p g f")
                dst = xtile.rearrange("p (g f) -> p g f", g=n)
                nc.sync.dma_start(out=dst, in_=src)
                for j, g in enumerate(grp):
                    block_slice[g] = (xtile, j * C * GS)
            ob = opool.tile([GS, BPC * GS], mybir.dt.float32, tag="o")
            NP = C // 2  # 42 chunk pairs
            for g in range(BPC):
                xtile, base = block_slice[g]
                # Chunk-pair matmul: stationary AND moving are the [128, 128]
                # pair of adjacent contraction chunks -> one LDWEIGHTS + one
                # MATMUL per two chunks. The [128,128] product holds the
                # even-chunk gram in quadrant [0:64, 0:64] and the odd-chunk
                # gram in [64:128, 64:128]; the off-diagonal quadrants are
                # discarded cross-chunk garbage.
                ps = ppool.tile([2 * GS, 2 * GS], mybir.dt.float32, tag="ps")
                for p in range(NP):
                    pair = xtile[:, base + p * 2 * GS: base + (p + 1) * 2 * GS]
                    nc.tensor.matmul(ps, lhsT=pair, rhs=pair,
                                     start=(p == 0), stop=(p == NP - 1))
                # gram = even-chunk sum + odd-chunk sum; input is pre-scaled
                # by 1/sqrt(L) on the host so no further scaling is needed.
                # Two DVE ops because only one operand may come from PSUM.
                oslice = ob[:, g * GS:(g + 1) * GS]
                nc.vector.tensor_copy(oslice, ps[0:GS, 0:GS])
                nc.vector.scalar_tensor_tensor(
                    oslice,
                    oslice,
                    1.0,
                    ps[GS:2 * GS, GS:2 * GS],
                    op0=mybir.AluOpType.mult,
                    op1=mybir.AluOpType.add,
                )
            nc.sync.dma_start(out=sims[:], in_=ob)

    nc.finalize()
    _CACHE["nc"] = nc
    return nc


def _build_nc_raw():
    """Raw-bass variant: fine-grained streaming with manual semaphores.

    16 half-block input DMAs all increment ONE semaphore (HWDGE completes
    them in FIFO order), so the PE waits at half-block granularity and never
    idles long enough for the HAM clock gate to re-throttle. Two PSUM
    accumulators rotate per block; DVE combines the even/odd gram quadrants
    into the output tile; a single output DMA drains it at the end.
    """
    if "nc" in _CACHE:
        return _CACHE["nc"]
    import concourse.mybir as mybir
    from concourse import bacc

    NP = C // 2                   # 42 chunk-pairs per block
    # Per-block input pieces, in chunk-pair units. Block 0 starts with small
    # pieces so the PE warms up as early as possible.
    PIECES0 = [10, 11, 21]
    PIECESN = [21, 21]
    pieces = []                   # (g, pair_start, n_pairs) per input DMA
    for g in range(BPC):
        q0 = 0
        for n in (PIECES0 if g == 0 else PIECESN):
            pieces.append((g, q0, n))
            q0 += n
    NDMA_IN = len(pieces)

    nc = bacc.Bacc("TRN2", target_bir_lowering=False)
    xt = nc.declare_dram_parameter("xt", [BPC, K, C * GS], mybir.dt.float16, isOutput=False)
    sims = nc.declare_dram_parameter("sims", [GS, BPC * GS], mybir.dt.float32, isOutput=True)

    from contextlib import ExitStack
    with ExitStack() as ctx:
        xb = [ctx.enter_context(nc.sbuf_tensor(f"xb{g}", [K, C * GS], mybir.dt.float16))
              for g in range(BPC)]
        ob = ctx.enter_context(nc.sbuf_tensor("ob", [GS, BPC * GS], mybir.dt.float32))
        ps = [ctx.enter_context(nc.psum_tensor(f"ps{i}", [2 * GS, 2 * GS], mybir.dt.float32))
              for i in range(2)]
        # One sem per input DMA: a DMA's +16 completion arrives as 16
        # separate +1s (one per SDMA engine), and engines skew across
        # pipelined DMAs, so a single shared counter is unsound.
        dma_sems = [ctx.enter_context(nc.semaphore(f"dma_in{h}"))
                    for h in range(NDMA_IN)]
        pe_sem = ctx.enter_context(nc.semaphore("pe_done"))
        dve_sem = ctx.enter_context(nc.semaphore("dve_done"))
        out_sem = ctx.enter_context(nc.semaphore("dma_out"))
        all_sems = [*dma_sems, pe_sem, dve_sem, out_sem]

        # Piece index per block for the PE's waits.
        by_block = {g: [] for g in range(BPC)}
        for h, (g, q0, n) in enumerate(pieces):
            by_block[g].append((h, q0, n))

        with nc.Block(no_gpsimd_drain=True) as block:

            @block.sync
            def _(sync):
                for h, (g, q0, n) in enumerate(pieces):
                    src = xt[g][:, q0 * 2 * GS:(q0 + n) * 2 * GS]
                    dst = xb[g][:, q0 * 2 * GS:(q0 + n) * 2 * GS]
                    sync.dma_start(out=dst, in_=src).then_inc(dma_sems[h], 16)
                sync.wait_ge(out_sem, 16 * BPC)
                # Leave all sems at 0 so the NEFF can be re-executed.
                nums = sorted(s.num for s in all_sems)
                if nums == list(range(nums[0], nums[0] + len(nums))):
                    sync.sem_clear(range(nums[0], nums[-1] + 1))
                else:
                    for s in all_sems:
                        sync.sem_clear(s)

            @block.tensor
            def _(tensor):
                for g in range(BPC):
                    p = ps[g % 2]
                    if g >= 2:
                        # PSUM WAR: block g-2's epilogue must have read this
                        # accumulator before we clear it with start=True.
                        tensor.wait_ge(dve_sem, g - 1)
                    mm = None
                    for h, q0, n in by_block[g]:
                        tensor.wait_ge(dma_sems[h], 16)
                        for q in range(q0, q0 + n):
                            pair = xb[g][:, q * 2 * GS:(q + 1) * 2 * GS]
                            mm = nc.tensor.matmul(
                                p[:], lhsT=pair, rhs=pair,
                                start=(q == 0), stop=(q == NP - 1))
                    mm.then_inc(pe_sem, 1)

            @block.vector
            def _(vector):
                for g in range(BPC):
                    p = ps[g % 2]
                    vector.wait_ge(pe_sem, g + 1)
                    oslice = ob[:, g * GS:(g + 1) * GS]
                    nc.vector.tensor_copy(oslice, p[0:GS, 0:GS])
                    nc.vector.scalar_tensor_tensor(
                        oslice, oslice, 1.0, p[GS:2 * GS, GS:2 * GS],
                        op0=mybir.AluOpType.mult,
                        op1=mybir.AluOpType.add,
                    ).then_inc(dve_sem, 1)

            @block.scalar
            def _(scalar):
                # Per-block result DMAs on the ACT HWDGE queue: only the last
                # block's 16KB store sits on the critical path.
                for g in range(BPC):
                    scalar.wait_ge(dve_sem, g + 1)
                    scalar.dma_start(
                        out=sims[:, g * GS:(g + 1) * GS],
                        in_=ob[:, g * GS:(g + 1) * GS],
                    ).then_inc(out_sem, 16)

        nc.finalize()
    _CACHE["nc"] = nc
    return nc


def _marshal(x):
    # [B,N,L,A] -> blocks [NBLK, K, C*GS] fp16, feature-major per block.
    # Pre-scaled by 1/sqrt(L) so the on-device gram needs no 1/L scaling.
    xs = np.asarray(x, dtype=np.float32).reshape(B, G, GS, C, K)
    xs = xs * np.float32(1.0 / np.sqrt(L))
    xt = xs.transpose(0, 1, 4, 3, 2)                 # [b, g, p, c, n]
    return np.ascontiguousarray(xt, dtype=np.float16).reshape(NBLK, K, C * GS)


def run(x, trace=False):
    import os
    from concourse.bass_utils import run_bass_kernel_spmd

    nc = _build_nc() if os.environ.get("HS_TILE_KERNEL") else _build_nc_raw()
    xt16 = _marshal(x)
    in_maps = [{"xt": xt16[m * BPC:(m + 1) * BPC]} for m in range(NCORES)]
    res = run_bass_kernel_spmd(nc, in_maps, list(range(NCORES)), trace=trace)
    # per-core result is [GS, BPC*GS] = [i, (g j)] -> [g, i, j]
    sims = np.stack(
        [
            res.results[m]["sims"].reshape(GS, BPC, GS).transpose(1, 0, 2)
            for m in range(NCORES)
        ],
    ).reshape(B, G, GS, GS)

    out = np.full((B, N, N), np.nan, dtype=np.float32)
    for g in range(G):
        out[:, g * GS:(g + 1) * GS, g * GS:(g + 1) * GS] = sims[:, g]
    return out, res


def kernel(x):
    out, _ = run(x, trace=False)
    return out
